# revision 7
# baseline (speedup 1.0000x reference)
"""Causal multi-head self-attention on 8 TRN2 NeuronCores.

Problem (hardcoded): x [4, 2048, 1024] f32, qkv_w [1024, 3072], proj_w
[1024, 1024], proj_b [1024], 16 heads of dim 64, causal softmax.

Sharding: core c handles batch b = c // 2 and head-half c % 2 (8 of the 16
heads). Each core computes the QKV projection for its 8 heads, causal
attention, and the partial output projection (its 512 rows of proj_w). The
host sums the two partials per batch and adds the bias.

All matmul operands are bf16 (accumulation in f32 PSUM): halves DMA bytes
and enables FWL so LDWEIGHTS hides under streaming. The host pre-permutes
each weight so it lands in SBUF with a single large contiguous DMA.

On-core dataflow (head-dim on partitions everywhere):
  qT/kT = W.T @ x.T  (bf16 matmuls, stored bf16)            [128, N] per pair
  v     = x @ Wv     (bf16, stored bf16, k-rows on parts)   [N, 512+ones]
  scoresT[k,q] per head = kT-slice.T @ qT   (row-tiled e0/e1 concurrent)
  expT  = exp(0.125 * scoresT) on ACT, tri-mask on the diagonal 128-block
  outT_unnorm[dh,q] += v-slice.T @ expT     (accumulated over k chunks)
  sums ride the v ones-column -> sel-matmul broadcast -> fast reciprocal
  outT  = po * recip fused on DVE (divide folded into the PSUM->SBUF copy)
  partial = outT.T @ proj_w (bf16)

Scheduling: the attention inner loop is paced by the ACT engine (exp), so
scores are emitted one k-block ahead of the AV matmuls (ACT never starves
behind the in-order PE queue), and the PE bubbles are filled with the next
pair's QKV-projection matmuls / the output projection, emitted as small
4-matmul closures one per k-block iteration.
"""

import numpy as np

P = 128
N = 2048
D = 1024
DH = 512          # head dims per core (8 heads x 64)
HD = 64
HP = HD + 1       # head dims + ones column (softmax denominator row)
DHP = 8 * HP      # per-row-chunk v columns incl. ones (520)
NPAIR = 4
DC = D // P       # 8 contraction chunks
NRC = N // P      # 16 row chunks
NQC = N // 512    # 4 query 512-chunks

_CACHE = {}


def _build_nc(reps=1):
    from collections import deque
    from contextlib import ExitStack

    import concourse.bacc as bacc
    import concourse.tile as tile
    from concourse import mybir

    f32 = mybir.dt.float32
    bf16 = mybir.dt.bfloat16
    AF = mybir.ActivationFunctionType

    nc = bacc.Bacc("TRN2", target_bir_lowering=False, debug=False,
                   enable_asserts=False, num_devices=8)

    # host-side layouts (see make_in_maps):
    #   xt  [128, 16 * 1024]: block (qw, dc) of 512 cols = x.T[dc-rows, qw-cols]
    #   wv/wq/wk [128, 8 * 512]: block dc = W[dc-rows, :]
    #   pw  [128, 4 * 1024]: block pp = proj_w[pp-rows, :]
    xt = nc.dram_tensor("xt", [P, 4 * 4096], bf16, kind="ExternalInput").ap()
    wq = nc.dram_tensor("wq", [P, DC * DH], bf16, kind="ExternalInput").ap()
    wk = nc.dram_tensor("wk", [P, DC * DH], bf16, kind="ExternalInput").ap()
    wv = nc.dram_tensor("wv", [P, DC * DH], bf16, kind="ExternalInput").ap()
    pw = nc.dram_tensor("pw", [P, NPAIR * D], bf16, kind="ExternalInput").ap()
    tri = nc.dram_tensor("tri", [P, P], bf16, kind="ExternalInput").ap()
    sel = nc.dram_tensor("sel", [P, P], bf16, kind="ExternalInput").ap()
    out = nc.dram_tensor("out", [N, D], f32, kind="ExternalOutput").ap()

    def emit_rep(tc, const_tiles):
        tri_b, sel_b = const_tiles
        with ExitStack() as rep:
            big_ps = rep.enter_context(
                tc.tile_pool(name="big_ps", bufs=2, space="PSUM"))
            small_ps = rep.enter_context(
                tc.tile_pool(name="small_ps", bufs=2, space="PSUM"))
            po_ps = rep.enter_context(
                tc.tile_pool(name="po_ps", bufs=1, space="PSUM"))
            outT_pool = rep.enter_context(tc.tile_pool(name="outT", bufs=4))
            outT = [outT_pool.tile([P, N], bf16, name=f"outT{p}", tag="outT")
                    for p in range(NPAIR)]
            pw_pool = rep.enter_context(tc.tile_pool(name="pw", bufs=1))
            osb = rep.enter_context(tc.tile_pool(name="osb", bufs=4))
            pw_sb = pw_pool.tile([P, NPAIR * D], bf16)

            with ExitStack() as mid:
                xt_pool = mid.enter_context(tc.tile_pool(name="xt", bufs=1))
                xt_sb = xt_pool.tile([P, 4 * 4096], bf16)

                def xts(dc, q0, w):
                    # columns q0:q0+w of x.T row-block dc; must not cross a
                    # 512-col boundary
                    qw, r = q0 // 512, q0 % 512
                    base = qw * 4096 + dc * 512 + r
                    return xt_sb[:, base:base + w]

                vt_pool = mid.enter_context(tc.tile_pool(name="vt", bufs=1))
                # per head: 64 v-columns + a ones column, so the AV matmul's
                # 65th output partition accumulates the softmax denominator
                v_sb = vt_pool.tile([P, NRC * DHP], bf16)
                w_pool = mid.enter_context(tc.tile_pool(name="wqkv", bufs=3))
                wv_sb = w_pool.tile([P, DC * DH], bf16, tag="wv")
                wq_sb = w_pool.tile([P, DC * DH], bf16, tag="wq")
                wk_sb = w_pool.tile([P, DC * DH], bf16, tag="wk")

                # weights on the sync queue, xt on the Act queue, pw/consts
                # on the gpsimd queue. The first pieces are fine-grained so
                # phase B2's first matmuls wait on ~128KB, not ~1MB.
                nc.sync.dma_start(wv_sb[:, 0:DH], wv[:, 0:DH])
                nc.sync.dma_start(wv_sb[:, DH:DC * DH], wv[:, DH:DC * DH])
                nc.sync.dma_start(wq_sb[:], wq)
                nc.sync.dma_start(wk_sb[:], wk)
                for dc in range(DC):
                    nc.scalar.dma_start(
                        xt_sb[:, dc * 512:(dc + 1) * 512],
                        xt[:, dc * 512:(dc + 1) * 512])
                for qw in range(1, 4):
                    nc.scalar.dma_start(
                        xt_sb[:, qw * 4096:(qw + 1) * 4096],
                        xt[:, qw * 4096:(qw + 1) * 4096])
                nc.gpsimd.dma_start(pw_sb[:], pw)

                # ---- Phase B2: v = x @ Wv (k-rows on partitions), bf16 ----
                for rc in range(NRC):
                    pv = small_ps.tile([P, DH], f32, tag="sp")
                    for dc in range(DC):
                        nc.tensor.matmul(
                            pv[:],
                            xts(dc, rc * P, P),
                            wv_sb[:, dc * DH:(dc + 1) * DH],
                            start=(dc == 0), stop=(dc == DC - 1))
                    dst = v_sb[:, rc * DHP:(rc + 1) * DHP].rearrange(
                        "p (h c) -> p h c", h=8)
                    nc.vector.tensor_copy(
                        dst[:, :, 0:HD],
                        pv[:].rearrange("p (h c) -> p h c", h=8))
                    nc.vector.memset(dst[:, :, HD:HP], 1.0)

                qkT = mid.enter_context(tc.tile_pool(name="qkT", bufs=2))
                expp = mid.enter_context(tc.tile_pool(name="expp", bufs=5))
                ssbp = mid.enter_context(tc.tile_pool(name="ssb", bufs=2))
                drc_pool = mid.enter_context(tc.tile_pool(name="drc", bufs=1))
                # two persistent divisor-staging tiles; rows 0/32 are
                # rewritten with the raw softmax sums each round, other rows
                # only need to be non-NaN for the sel matmul
                ssb_t = []
                for i in range(2):
                    t2 = ssbp.tile([P, 512], bf16, name=f"ssbt{i}", tag="ssb")
                    nc.vector.memset(t2[0:HD, :], 1.0)
                    ssb_t.append(t2)

                def qkT_chunk_closures(p, qc, wt, dstT):
                    """One 512-col chunk of the q or k projection for pair p,
                    split into two 4-matmul filler closures (the second also
                    casts PSUM -> SBUF)."""
                    state = {}

                    def first():
                        state["ps"] = small_ps.tile([P, 512], f32,
                                                    name="fqk", tag="sp")
                        for dc in range(4):
                            nc.tensor.matmul(
                                state["ps"][:],
                                wt[:, dc * DH + p * P: dc * DH + (p + 1) * P],
                                xts(dc, qc * 512, 512),
                                start=(dc == 0), stop=False,
                                skip_group_check=True)

                    def second():
                        for dc in range(4, DC):
                            nc.tensor.matmul(
                                state["ps"][:],
                                wt[:, dc * DH + p * P: dc * DH + (p + 1) * P],
                                xts(dc, qc * 512, 512),
                                start=False, stop=(dc == DC - 1),
                                skip_group_check=True)
                        nc.vector.tensor_copy(
                            dstT[:, qc * 512:(qc + 1) * 512], state["ps"][:])

                    return [first, second]

                def proj_closure(rc, cc):
                    """Output projection for one [128, 512] tile: 4
                    accumulation matmuls + PSUM cast + store."""
                    def go():
                        pr = small_ps.tile([P, 512], f32, name="pr", tag="sp")
                        for pp in range(NPAIR):
                            nc.tensor.matmul(
                                pr[:],
                                outT[pp][:, rc * P:(rc + 1) * P],
                                pw_sb[:, pp * D + cc * 512:
                                      pp * D + (cc + 1) * 512],
                                start=(pp == 0), stop=(pp == NPAIR - 1))
                        ot = osb.tile([P, 512], f32, name="ot", tag="osb")
                        nc.vector.tensor_copy(ot[:], pr[:])
                        nc.gpsimd.dma_start(
                            out[rc * P:(rc + 1) * P,
                                cc * 512:(cc + 1) * 512], ot[:])
                    return go

                # ---- qkT for pair 0 (upfront; later pairs are filler) ----
                qT = qkT.tile([P, N], bf16, tag="qT")
                kT = qkT.tile([P, N], bf16, tag="kT")
                for qc in range(NQC):
                    for c in qkT_chunk_closures(0, qc, wq_sb, qT):
                        c()
                    for c in qkT_chunk_closures(0, qc, wk_sb, kT):
                        c()

                for p in range(NPAIR):
                    fillq = deque()
                    if p + 1 < NPAIR:
                        qT_n = qkT.tile([P, N], bf16, tag="qT")
                        kT_n = qkT.tile([P, N], bf16, tag="kT")
                        for qc in range(NQC):
                            fillq.extend(
                                qkT_chunk_closures(p + 1, qc, wq_sb, qT_n))
                            fillq.extend(
                                qkT_chunk_closures(p + 1, qc, wk_sb, kT_n))

                    # ---- Attention for this pair: one flat k-block stream;
                    # scores are emitted one step ahead (across q-window
                    # boundaries too) so the ACT exp stream never starves
                    # behind the in-order PE queue ------------------------
                    seq = [(qc4, kc) for qc4 in range(NQC)
                           for kc in range(4 * qc4 + 4)]

                    def emit_scores(qc4, kc):
                        qoff = max(0, kc * P - qc4 * 512)
                        q0 = qc4 * 512 + qoff
                        q1 = (qc4 + 1) * 512
                        ps_s = big_ps.tile([P, 1024], f32, name="ps_s",
                                           tag="bp")
                        for e in range(2):
                            nc.tensor.matmul(
                                ps_s[:, e * 512 + qoff: e * 512 + 512],
                                kT[e * HD:(e + 1) * HD,
                                   kc * P:(kc + 1) * P],
                                qT[e * HD:(e + 1) * HD, q0:q1],
                                start=True, stop=True)
                        return ps_s, qoff

                    po = None
                    ps_cur = emit_scores(*seq[0])
                    for i, (qc4, kc) in enumerate(seq):
                        nkc = 4 * qc4 + 4
                        ps_s, qoff = ps_cur
                        ps_cur = emit_scores(*seq[i + 1]) \
                            if i + 1 < len(seq) else None
                        et = expp.tile([P, 1024], bf16, name="et", tag="et")
                        ev = et[:].rearrange(
                            "p (h q) -> p h q", h=2)[:, :, qoff:512]
                        pv_ = ps_s[:].rearrange(
                            "p (h q) -> p h q", h=2)[:, :, qoff:512]
                        nc.scalar.activation(ev, pv_, AF.Exp, scale=0.125)
                        if kc >= 4 * qc4:  # diagonal block: causal mask
                            em = et[:].rearrange("p (a q) -> p a q", a=2)[
                                :, :, qoff:qoff + P]
                            trib = tri_b[:].rearrange(
                                "p (a q) -> p a q", a=1).broadcast_to(
                                [P, 2, P])
                            nc.vector.tensor_mul(em, em, trib)
                        if kc == 0:
                            po = [po_ps.tile([HP, 512], f32, name=f"po{e}",
                                             tag=f"po{e}") for e in range(2)]
                        for e in range(2):
                            h = 2 * p + e
                            nc.tensor.matmul(
                                po[e][0:HP, qoff:512],
                                v_sb[:, kc * DHP + h * HP:
                                     kc * DHP + (h + 1) * HP],
                                et[:, e * 512 + qoff: e * 512 + 512],
                                start=(kc == 0), stop=(kc == nkc - 1),
                                skip_group_check=True)
                        if fillq:
                            fillq.popleft()()
                        if kc == nkc - 1:
                            # q-window tail: drain po quickly (sums + raw
                            # copies), then the divisor chain and a single
                            # in-place normalize of the outT slice
                            qs = slice(qc4 * 512, (qc4 + 1) * 512)
                            ssb = ssb_t[(p * NQC + qc4) % 2]
                            nc.vector.tensor_copy(ssb[0:1, :],
                                                  po[0][HD:HP, :])
                            nc.vector.tensor_copy(ssb[32:33, :],
                                                  po[1][HD:HP, :])
                            nc.vector.tensor_copy(outT[p][0:HD, qs],
                                                  po[0][0:HD, :])
                            nc.vector.tensor_copy(outT[p][HD:P, qs],
                                                  po[1][0:HD, :])
                            dps = small_ps.tile([P, 512], f32, name="dps",
                                                tag="sp")
                            nc.tensor.matmul(dps[:], sel_b[0:33, :],
                                             ssb[0:33, :], start=True,
                                             stop=True)
                            drc = drc_pool.tile([P, 512], f32, tag="drc")
                            nc.vector.reciprocal_approx_fast(drc[:], dps[:])
                            nc.vector.tensor_mul(outT[p][:, qs],
                                                 outT[p][:, qs], drc[:])
                            if p == NPAIR - 1:
                                for rc in range(4 * qc4, 4 * qc4 + 4):
                                    for cc in range(2):
                                        fillq.append(proj_closure(rc, cc))
                            if fillq:
                                fillq.popleft()()

                    # drain leftover fillers before the next pair needs them
                    while fillq:
                        fillq.popleft()()
                    if p + 1 < NPAIR:
                        qT, kT = qT_n, kT_n

    with tile.TileContext(nc) as tc, ExitStack() as ctx:
        const = ctx.enter_context(tc.tile_pool(name="const", bufs=1))
        tri_b = const.tile([P, P], bf16)
        nc.gpsimd.dma_start(tri_b[:], tri)
        sel_b = const.tile([P, P], bf16)
        nc.gpsimd.dma_start(sel_b[:], sel)
        const_tiles = (tri_b, sel_b)
        for _rep in range(reps):
            emit_rep(tc, const_tiles)

    nc.compile()
    return nc


def get_nc(reps=1):
    key = f"nc{reps}"
    if key not in _CACHE:
        _CACHE[key] = _build_nc(reps=reps)
    return _CACHE[key]


def _make_runner(nc, n_cores=8):
    """Cached jit over the bass_exec primitive (mirrors
    bass2jax.run_bass_via_pjrt's multi-core path, but reusable across calls
    so jax does not re-trace per invocation)."""
    import jax
    from jax.sharding import Mesh, PartitionSpec
    from jax.experimental.shard_map import shard_map
    from concourse import bass2jax, mybir

    bass2jax.install_neuronx_cc_hook()
    part_name = nc.partition_id_tensor.name if nc.partition_id_tensor else None
    in_names, out_names, out_avals, zero_templates = [], [], [], []
    for alloc in nc.m.functions[0].allocations:
        if not isinstance(alloc, mybir.MemoryLocationSet):
            continue
        name = alloc.memorylocations[0].name
        if alloc.kind == "ExternalInput":
            if name != part_name:
                in_names.append(name)
        elif alloc.kind == "ExternalOutput":
            out_names.append(name)
            shape = tuple(alloc.tensor_shape)
            dtype = mybir.dt.np(alloc.dtype)
            out_avals.append(jax.core.ShapedArray(shape, dtype))
            zero_templates.append((shape, dtype))
    n_params = len(in_names)
    n_outs = len(out_avals)
    all_names = in_names + out_names + ([part_name] if part_name else [])

    def _body(*args):
        operands = list(args)
        if part_name:
            operands.append(bass2jax.partition_id_tensor())
        outs = bass2jax._bass_exec_p.bind(
            *operands,
            out_avals=tuple(out_avals),
            in_names=tuple(all_names),
            out_names=tuple(out_names),
            lowering_input_output_aliases=(),
            sim_require_finite=True,
            sim_require_nnan=True,
            nc=nc,
        )
        return tuple(outs)

    devices = jax.devices()[:n_cores]
    mesh = Mesh(np.asarray(devices), ("core",))
    in_specs = (PartitionSpec("core"),) * (n_params + n_outs)
    out_specs = (PartitionSpec("core"),) * n_outs
    donate = tuple(range(n_params, n_params + n_outs))
    sharded = jax.jit(
        shard_map(_body, mesh=mesh, in_specs=in_specs, out_specs=out_specs,
                  check_rep=False),
        donate_argnums=donate, keep_unused=True)

    def run(in_maps):
        concat_in = [
            np.concatenate([np.asarray(m[name]) for m in in_maps], axis=0)
            for name in in_names
        ]
        concat_zeros = [
            np.zeros((n_cores * s[0], *s[1:]), d) for s, d in zero_templates
        ]
        out_arrs = sharded(*concat_in, *concat_zeros)
        return {
            name: np.asarray(out_arrs[i]).reshape(n_cores, *zero_templates[i][0])
            for i, name in enumerate(out_names)
        }

    run.sharded = sharded
    run.mesh = mesh
    run.in_names = in_names
    run.out_names = out_names
    run.zero_templates = zero_templates
    run.n_cores = n_cores
    return run


def get_runner(reps=1):
    key = f"runner{reps}"
    if key not in _CACHE:
        _CACHE[key] = _make_runner(get_nc(reps=reps))
    return _CACHE[key]


def _fold_rows(a):
    """[8*128, C] -> [128, 8*C]: row-block dc becomes column-block dc."""
    dcn, c = a.shape[0] // P, a.shape[1]
    return np.ascontiguousarray(
        a.reshape(dcn, P, c).transpose(1, 0, 2).reshape(P, dcn * c))


def make_in_maps(x, qkv_w, proj_w):
    import ml_dtypes
    bf = ml_dtypes.bfloat16
    x = np.asarray(x, dtype=np.float32)
    qkv_w = np.asarray(qkv_w, dtype=bf)
    proj_w = np.asarray(proj_w, dtype=bf)
    tri = np.triu(np.ones((P, P), dtype=bf))
    sel = np.zeros((P, P), dtype=bf)
    sel[0, 0:64] = 1.0
    sel[32, 64:128] = 1.0
    in_maps = []
    for c in range(8):
        b, half = c // 2, c % 2
        hs = half * DH
        xtb = x[b].T.astype(bf)  # [1024, 2048]
        # [p, (qw dc c)]: 512-col chunk of x.T row-block dc, query window qw
        xtr = np.ascontiguousarray(
            xtb.reshape(DC, P, 4, 512).transpose(1, 2, 0, 3).reshape(
                P, 4 * 4096))
        in_maps.append({
            "xt": xtr,
            "wq": _fold_rows(qkv_w[:, hs:hs + DH]),
            "wk": _fold_rows(qkv_w[:, D + hs:D + hs + DH]),
            "wv": _fold_rows(qkv_w[:, 2 * D + hs:2 * D + hs + DH]),
            "pw": _fold_rows(proj_w[hs:hs + DH, :]),
            "tri": tri,
            "sel": sel,
        })
    return in_maps


def kernel(x, qkv_w, proj_w, proj_b, **_):
    proj_b = np.asarray(proj_b, dtype=np.float32)
    run = get_runner()
    in_maps = make_in_maps(x, qkv_w, proj_w)
    parts = run(in_maps)["out"]
    outp = np.empty((4, N, D), dtype=np.float32)
    for b in range(4):
        outp[b] = parts[2 * b] + parts[2 * b + 1] + proj_b[None, :]
    return outp


# revision 10
# speedup vs baseline: 1.0112x; 1.0112x over previous
"""Causal multi-head self-attention on 8 TRN2 NeuronCores.

Problem (hardcoded): x [4, 2048, 1024] f32, qkv_w [1024, 3072], proj_w
[1024, 1024], proj_b [1024], 16 heads of dim 64, causal softmax.

Sharding: core c handles batch b = c // 2 and head-half c % 2 (8 of the 16
heads). Each core computes the QKV projection for its 8 heads, causal
attention, and the partial output projection (its 512 rows of proj_w). The
host sums the two partials per batch and adds the bias.

All matmul operands are bf16 (accumulation in f32 PSUM): halves DMA bytes
and enables FWL so LDWEIGHTS hides under streaming. The host pre-permutes
each weight so it lands in SBUF with a single large contiguous DMA.

On-core dataflow (head-dim on partitions everywhere):
  qT/kT = W.T @ x.T  (bf16 matmuls, stored bf16)            [128, N] per pair
  v     = x @ Wv     (bf16, stored bf16, k-rows on parts)   [N, 512+ones]
  scoresT[k,q] per head = kT-slice.T @ qT   (row-tiled e0/e1 concurrent)
  expT  = exp(0.125 * scoresT) on ACT, tri-mask on the diagonal 128-block
  outT_unnorm[dh,q] += v-slice.T @ expT     (accumulated over k chunks)
  sums ride the v ones-column -> sel-matmul broadcast -> fast reciprocal
  outT  = po * recip fused on DVE (divide folded into the PSUM->SBUF copy)
  partial = outT.T @ proj_w (bf16)

Scheduling: the attention inner loop is paced by the ACT engine (exp), so
scores are emitted one k-block ahead of the AV matmuls (ACT never starves
behind the in-order PE queue), and the PE bubbles are filled with the next
pair's QKV-projection matmuls / the output projection, emitted as small
4-matmul closures one per k-block iteration.
"""

import numpy as np

P = 128
N = 2048
D = 1024
DH = 512          # head dims per core (8 heads x 64)
HD = 64
HP = HD + 1       # head dims + ones column (softmax denominator row)
DHP = 8 * HP      # per-row-chunk v columns incl. ones (520)
NPAIR = 4
DC = D // P       # 8 contraction chunks
NRC = N // P      # 16 row chunks
NQC = N // 512    # 4 query 512-chunks

_CACHE = {}


def _build_nc(reps=1):
    from collections import deque
    from contextlib import ExitStack

    import concourse.bacc as bacc
    import concourse.tile as tile
    from concourse import mybir

    f32 = mybir.dt.float32
    bf16 = mybir.dt.bfloat16
    AF = mybir.ActivationFunctionType

    nc = bacc.Bacc("TRN2", target_bir_lowering=False, debug=False,
                   enable_asserts=False, num_devices=8)

    # host-side layouts (see make_in_maps):
    #   xt  [128, 16 * 1024]: block (qw, dc) of 512 cols = x.T[dc-rows, qw-cols]
    #   wv/wq/wk [128, 8 * 512]: block dc = W[dc-rows, :]
    #   pw  [128, 4 * 1024]: block pp = proj_w[pp-rows, :]
    xt = nc.dram_tensor("xt", [P, 4 * 4096], bf16, kind="ExternalInput").ap()
    wq = nc.dram_tensor("wq", [P, DC * DH], bf16, kind="ExternalInput").ap()
    wk = nc.dram_tensor("wk", [P, DC * DH], bf16, kind="ExternalInput").ap()
    wv = nc.dram_tensor("wv", [P, DC * DH], bf16, kind="ExternalInput").ap()
    pw = nc.dram_tensor("pw", [P, NPAIR * D], bf16, kind="ExternalInput").ap()
    tri = nc.dram_tensor("tri", [P, P], bf16, kind="ExternalInput").ap()
    sel = nc.dram_tensor("sel", [P, P], bf16, kind="ExternalInput").ap()
    out = nc.dram_tensor("out", [N, D], f32, kind="ExternalOutput").ap()

    def emit_rep(tc, const_tiles):
        tri_b, sel_b = const_tiles
        with ExitStack() as rep:
            big_ps = rep.enter_context(
                tc.tile_pool(name="big_ps", bufs=2, space="PSUM"))
            small_ps = rep.enter_context(
                tc.tile_pool(name="small_ps", bufs=2, space="PSUM"))
            po_ps = rep.enter_context(
                tc.tile_pool(name="po_ps", bufs=1, space="PSUM"))
            outT_pool = rep.enter_context(tc.tile_pool(name="outT", bufs=4))
            outT = [outT_pool.tile([P, N], bf16, name=f"outT{p}", tag="outT")
                    for p in range(NPAIR)]
            pw_pool = rep.enter_context(tc.tile_pool(name="pw", bufs=1))
            osb = rep.enter_context(tc.tile_pool(name="osb", bufs=4))
            pw_sb = pw_pool.tile([P, NPAIR * D], bf16)

            with ExitStack() as mid:
                xt_pool = mid.enter_context(tc.tile_pool(name="xt", bufs=1))
                xt_sb = xt_pool.tile([P, 4 * 4096], bf16)

                def xts(dc, q0, w):
                    # columns q0:q0+w of x.T row-block dc; must not cross a
                    # 512-col boundary
                    qw, r = q0 // 512, q0 % 512
                    base = qw * 4096 + dc * 512 + r
                    return xt_sb[:, base:base + w]

                vt_pool = mid.enter_context(tc.tile_pool(name="vt", bufs=1))
                # per head: 64 v-columns + a ones column, so the AV matmul's
                # 65th output partition accumulates the softmax denominator
                v_sb = vt_pool.tile([P, NRC * DHP], bf16)
                w_pool = mid.enter_context(tc.tile_pool(name="wqkv", bufs=3))
                wv_sb = w_pool.tile([P, DC * DH], bf16, tag="wv")
                wq_sb = w_pool.tile([P, DC * DH], bf16, tag="wq")
                wk_sb = w_pool.tile([P, DC * DH], bf16, tag="wk")

                # weights on the sync queue, xt on the Act queue, pw/consts
                # on the gpsimd queue. The first pieces are fine-grained so
                # phase B2's first matmuls wait on ~128KB, not ~1MB.
                nc.sync.dma_start(wv_sb[:, 0:DH], wv[:, 0:DH])
                nc.sync.dma_start(wv_sb[:, DH:DC * DH], wv[:, DH:DC * DH])
                nc.sync.dma_start(wq_sb[:], wq)
                nc.sync.dma_start(wk_sb[:], wk)
                # xt splits across the Act and gpsimd DMA queues (~160GB/s
                # each) so phase B2 is not gated on a single queue; the
                # gpsimd queue is otherwise idle until the output stores
                for dc in range(DC):
                    nc.scalar.dma_start(
                        xt_sb[:, dc * 512:(dc + 1) * 512],
                        xt[:, dc * 512:(dc + 1) * 512])
                for qw, eng in ((1, nc.gpsimd), (2, nc.scalar),
                                (3, nc.gpsimd)):
                    lo = qw * 4096
                    eng.dma_start(xt_sb[:, lo:lo + 4096],
                                  xt[:, lo:lo + 4096])
                nc.gpsimd.dma_start(pw_sb[:], pw)

                # ---- Phase B2: v = x @ Wv (k-rows on partitions), bf16 ----
                for rc in range(NRC):
                    pv = small_ps.tile([P, DH], f32, tag="sp")
                    for dc in range(DC):
                        nc.tensor.matmul(
                            pv[:],
                            xts(dc, rc * P, P),
                            wv_sb[:, dc * DH:(dc + 1) * DH],
                            start=(dc == 0), stop=(dc == DC - 1))
                    dst = v_sb[:, rc * DHP:(rc + 1) * DHP].rearrange(
                        "p (h c) -> p h c", h=8)
                    nc.vector.tensor_copy(
                        dst[:, :, 0:HD],
                        pv[:].rearrange("p (h c) -> p h c", h=8))
                    nc.vector.memset(dst[:, :, HD:HP], 1.0)

                qkT = mid.enter_context(tc.tile_pool(name="qkT", bufs=2))
                expp = mid.enter_context(tc.tile_pool(name="expp", bufs=5))
                ssbp = mid.enter_context(tc.tile_pool(name="ssb", bufs=2))
                drc_pool = mid.enter_context(tc.tile_pool(name="drc", bufs=1))
                # two persistent divisor-staging tiles; rows 0/32 are
                # rewritten with the raw softmax sums each round, other rows
                # only need to be non-NaN for the sel matmul
                ssb_t = []
                for i in range(2):
                    t2 = ssbp.tile([P, 512], bf16, name=f"ssbt{i}", tag="ssb")
                    nc.vector.memset(t2[0:HD, :], 1.0)
                    ssb_t.append(t2)

                def qkT_chunk_closures(p, qc, wt, dstT):
                    """One 512-col chunk of the q or k projection for pair p,
                    split into two 4-matmul filler closures (the second also
                    casts PSUM -> SBUF)."""
                    state = {}

                    def first():
                        state["ps"] = small_ps.tile([P, 512], f32,
                                                    name="fqk", tag="sp")
                        for dc in range(4):
                            nc.tensor.matmul(
                                state["ps"][:],
                                wt[:, dc * DH + p * P: dc * DH + (p + 1) * P],
                                xts(dc, qc * 512, 512),
                                start=(dc == 0), stop=False,
                                skip_group_check=True)

                    def second():
                        for dc in range(4, DC):
                            nc.tensor.matmul(
                                state["ps"][:],
                                wt[:, dc * DH + p * P: dc * DH + (p + 1) * P],
                                xts(dc, qc * 512, 512),
                                start=False, stop=(dc == DC - 1),
                                skip_group_check=True)
                        nc.vector.tensor_copy(
                            dstT[:, qc * 512:(qc + 1) * 512], state["ps"][:])

                    return [first, second]

                def proj_closure(rc, cc):
                    """Output projection for one [128, 512] tile: 4
                    accumulation matmuls + PSUM cast + store."""
                    def go():
                        pr = small_ps.tile([P, 512], f32, name="pr", tag="sp")
                        for pp in range(NPAIR):
                            nc.tensor.matmul(
                                pr[:],
                                outT[pp][:, rc * P:(rc + 1) * P],
                                pw_sb[:, pp * D + cc * 512:
                                      pp * D + (cc + 1) * 512],
                                start=(pp == 0), stop=(pp == NPAIR - 1))
                        ot = osb.tile([P, 512], f32, name="ot", tag="osb")
                        nc.vector.tensor_copy(ot[:], pr[:])
                        nc.gpsimd.dma_start(
                            out[rc * P:(rc + 1) * P,
                                cc * 512:(cc + 1) * 512], ot[:])
                    return go

                # ---- qkT for pair 0 (upfront; later pairs are filler) ----
                qkt = {0: (qkT.tile([P, N], bf16, name="qT0", tag="qT"),
                           qkT.tile([P, N], bf16, name="kT0", tag="kT"))}
                for qc in range(NQC):
                    for c in qkT_chunk_closures(0, qc, wq_sb, qkt[0][0]):
                        c()
                    for c in qkT_chunk_closures(0, qc, wk_sb, qkt[0][1]):
                        c()

                # ---- Attention: one flat global (pair, q-window, k-block)
                # stream. Scores run one step ahead and AV one step behind
                # the exp stream, so a blocked AV (waiting for the po bank
                # to drain at a window tail) never starves the ACT engine.
                gseq = [(p, qc4, kc) for p in range(NPAIR)
                        for qc4 in range(NQC) for kc in range(4 * qc4 + 4)]
                fillq = deque()
                po = [None, None]

                def emit_scores(p, qc4, kc):
                    qT, kT = qkt[p]
                    qoff = max(0, kc * P - qc4 * 512)
                    q0 = qc4 * 512 + qoff
                    q1 = (qc4 + 1) * 512
                    ps_s = big_ps.tile([P, 1024], f32, name="ps_s", tag="bp")
                    for e in range(2):
                        nc.tensor.matmul(
                            ps_s[:, e * 512 + qoff: e * 512 + 512],
                            kT[e * HD:(e + 1) * HD, kc * P:(kc + 1) * P],
                            qT[e * HD:(e + 1) * HD, q0:q1],
                            start=True, stop=True)
                    return ps_s, qoff

                def do_av(et, qoff, p, qc4, kc):
                    nkc = 4 * qc4 + 4
                    if kc == 0:
                        po[0] = po_ps.tile([HP, 512], f32, name="po0",
                                           tag="po0")
                        po[1] = po_ps.tile([HP, 512], f32, name="po1",
                                           tag="po1")
                    for e in range(2):
                        h = 2 * p + e
                        nc.tensor.matmul(
                            po[e][0:HP, qoff:512],
                            v_sb[:, kc * DHP + h * HP:
                                 kc * DHP + (h + 1) * HP],
                            et[:, e * 512 + qoff: e * 512 + 512],
                            start=(kc == 0), stop=(kc == nkc - 1),
                            skip_group_check=True)
                    if fillq:
                        fillq.popleft()()
                    if kc == nkc - 1:
                        # q-window tail: drain po quickly (sums + raw
                        # copies), then the divisor chain and one in-place
                        # normalize of the outT slice
                        qs = slice(qc4 * 512, (qc4 + 1) * 512)
                        ssb = ssb_t[(p * NQC + qc4) % 2]
                        nc.vector.tensor_copy(ssb[0:1, :], po[0][HD:HP, :])
                        nc.vector.tensor_copy(ssb[32:33, :], po[1][HD:HP, :])
                        nc.vector.tensor_copy(outT[p][0:HD, qs],
                                              po[0][0:HD, :])
                        nc.vector.tensor_copy(outT[p][HD:P, qs],
                                              po[1][0:HD, :])
                        dps = small_ps.tile([P, 512], f32, name="dps",
                                            tag="sp")
                        nc.tensor.matmul(dps[:], sel_b[0:33, :],
                                         ssb[0:33, :], start=True, stop=True)
                        drc = drc_pool.tile([P, 512], f32, tag="drc")
                        nc.vector.reciprocal_approx_fast(drc[:], dps[:])
                        nc.vector.tensor_mul(outT[p][:, qs],
                                             outT[p][:, qs], drc[:])
                        if p == NPAIR - 1:
                            for rc in range(4 * qc4, 4 * qc4 + 4):
                                for cc in range(2):
                                    fillq.append(proj_closure(rc, cc))
                        if fillq:
                            fillq.popleft()()

                sc_fifo = deque([emit_scores(*gseq[0])])
                prev_av = None
                cur_pair = -1
                for gi, it in enumerate(gseq):
                    p, qc4, kc = it
                    if p != cur_pair:
                        cur_pair = p
                        if p + 1 < NPAIR:
                            qkt[p + 1] = (
                                qkT.tile([P, N], bf16, name="qTn", tag="qT"),
                                qkT.tile([P, N], bf16, name="kTn", tag="kT"))
                            for qc in range(NQC):
                                fillq.extend(qkT_chunk_closures(
                                    p + 1, qc, wq_sb, qkt[p + 1][0]))
                                fillq.extend(qkT_chunk_closures(
                                    p + 1, qc, wk_sb, qkt[p + 1][1]))
                    if gi + 1 < len(gseq):
                        if gseq[gi + 1][0] != p:
                            # next pair's scores need its qkT casts emitted
                            while fillq:
                                fillq.popleft()()
                        sc_fifo.append(emit_scores(*gseq[gi + 1]))
                    ps_s, qoff = sc_fifo.popleft()
                    et = expp.tile([P, 1024], bf16, name="et", tag="et")
                    ev = et[:].rearrange(
                        "p (h q) -> p h q", h=2)[:, :, qoff:512]
                    pv_ = ps_s[:].rearrange(
                        "p (h q) -> p h q", h=2)[:, :, qoff:512]
                    nc.scalar.activation(ev, pv_, AF.Exp, scale=0.125)
                    if kc >= 4 * qc4:  # diagonal block: causal mask
                        em = et[:].rearrange("p (a q) -> p a q", a=2)[
                            :, :, qoff:qoff + P]
                        trib = tri_b[:].rearrange(
                            "p (a q) -> p a q", a=1).broadcast_to([P, 2, P])
                        nc.vector.tensor_mul(em, em, trib)
                    if prev_av is not None:
                        do_av(*prev_av)
                    prev_av = (et, qoff) + it
                do_av(*prev_av)
                while fillq:
                    fillq.popleft()()

    with tile.TileContext(nc) as tc, ExitStack() as ctx:
        const = ctx.enter_context(tc.tile_pool(name="const", bufs=1))
        tri_b = const.tile([P, P], bf16)
        nc.gpsimd.dma_start(tri_b[:], tri)
        sel_b = const.tile([P, P], bf16)
        nc.gpsimd.dma_start(sel_b[:], sel)
        const_tiles = (tri_b, sel_b)
        for _rep in range(reps):
            emit_rep(tc, const_tiles)

    nc.compile()
    return nc


def get_nc(reps=1):
    key = f"nc{reps}"
    if key not in _CACHE:
        _CACHE[key] = _build_nc(reps=reps)
    return _CACHE[key]


def _make_runner(nc, n_cores=8):
    """Cached jit over the bass_exec primitive (mirrors
    bass2jax.run_bass_via_pjrt's multi-core path, but reusable across calls
    so jax does not re-trace per invocation)."""
    import jax
    from jax.sharding import Mesh, PartitionSpec
    from jax.experimental.shard_map import shard_map
    from concourse import bass2jax, mybir

    bass2jax.install_neuronx_cc_hook()
    part_name = nc.partition_id_tensor.name if nc.partition_id_tensor else None
    in_names, out_names, out_avals, zero_templates = [], [], [], []
    for alloc in nc.m.functions[0].allocations:
        if not isinstance(alloc, mybir.MemoryLocationSet):
            continue
        name = alloc.memorylocations[0].name
        if alloc.kind == "ExternalInput":
            if name != part_name:
                in_names.append(name)
        elif alloc.kind == "ExternalOutput":
            out_names.append(name)
            shape = tuple(alloc.tensor_shape)
            dtype = mybir.dt.np(alloc.dtype)
            out_avals.append(jax.core.ShapedArray(shape, dtype))
            zero_templates.append((shape, dtype))
    n_params = len(in_names)
    n_outs = len(out_avals)
    all_names = in_names + out_names + ([part_name] if part_name else [])

    def _body(*args):
        operands = list(args)
        if part_name:
            operands.append(bass2jax.partition_id_tensor())
        outs = bass2jax._bass_exec_p.bind(
            *operands,
            out_avals=tuple(out_avals),
            in_names=tuple(all_names),
            out_names=tuple(out_names),
            lowering_input_output_aliases=(),
            sim_require_finite=True,
            sim_require_nnan=True,
            nc=nc,
        )
        return tuple(outs)

    devices = jax.devices()[:n_cores]
    mesh = Mesh(np.asarray(devices), ("core",))
    in_specs = (PartitionSpec("core"),) * (n_params + n_outs)
    out_specs = (PartitionSpec("core"),) * n_outs
    donate = tuple(range(n_params, n_params + n_outs))
    sharded = jax.jit(
        shard_map(_body, mesh=mesh, in_specs=in_specs, out_specs=out_specs,
                  check_rep=False),
        donate_argnums=donate, keep_unused=True)

    def run(in_maps):
        concat_in = [
            np.concatenate([np.asarray(m[name]) for m in in_maps], axis=0)
            for name in in_names
        ]
        concat_zeros = [
            np.zeros((n_cores * s[0], *s[1:]), d) for s, d in zero_templates
        ]
        out_arrs = sharded(*concat_in, *concat_zeros)
        return {
            name: np.asarray(out_arrs[i]).reshape(n_cores, *zero_templates[i][0])
            for i, name in enumerate(out_names)
        }

    run.sharded = sharded
    run.mesh = mesh
    run.in_names = in_names
    run.out_names = out_names
    run.zero_templates = zero_templates
    run.n_cores = n_cores
    return run


def get_runner(reps=1):
    key = f"runner{reps}"
    if key not in _CACHE:
        _CACHE[key] = _make_runner(get_nc(reps=reps))
    return _CACHE[key]


def _fold_rows(a):
    """[8*128, C] -> [128, 8*C]: row-block dc becomes column-block dc."""
    dcn, c = a.shape[0] // P, a.shape[1]
    return np.ascontiguousarray(
        a.reshape(dcn, P, c).transpose(1, 0, 2).reshape(P, dcn * c))


def make_in_maps(x, qkv_w, proj_w):
    import ml_dtypes
    bf = ml_dtypes.bfloat16
    x = np.asarray(x, dtype=np.float32)
    qkv_w = np.asarray(qkv_w, dtype=bf)
    proj_w = np.asarray(proj_w, dtype=bf)
    tri = np.triu(np.ones((P, P), dtype=bf))
    sel = np.zeros((P, P), dtype=bf)
    sel[0, 0:64] = 1.0
    sel[32, 64:128] = 1.0
    in_maps = []
    for c in range(8):
        b, half = c // 2, c % 2
        hs = half * DH
        xtb = x[b].T.astype(bf)  # [1024, 2048]
        # [p, (qw dc c)]: 512-col chunk of x.T row-block dc, query window qw
        xtr = np.ascontiguousarray(
            xtb.reshape(DC, P, 4, 512).transpose(1, 2, 0, 3).reshape(
                P, 4 * 4096))
        in_maps.append({
            "xt": xtr,
            "wq": _fold_rows(qkv_w[:, hs:hs + DH]),
            "wk": _fold_rows(qkv_w[:, D + hs:D + hs + DH]),
            "wv": _fold_rows(qkv_w[:, 2 * D + hs:2 * D + hs + DH]),
            "pw": _fold_rows(proj_w[hs:hs + DH, :]),
            "tri": tri,
            "sel": sel,
        })
    return in_maps


def kernel(x, qkv_w, proj_w, proj_b, **_):
    proj_b = np.asarray(proj_b, dtype=np.float32)
    run = get_runner()
    in_maps = make_in_maps(x, qkv_w, proj_w)
    parts = run(in_maps)["out"]
    outp = np.empty((4, N, D), dtype=np.float32)
    for b in range(4):
        outp[b] = parts[2 * b] + parts[2 * b + 1] + proj_b[None, :]
    return outp


# revision 17
# speedup vs baseline: 1.0585x; 1.0468x over previous
"""Causal multi-head self-attention on 8 TRN2 NeuronCores.

Problem (hardcoded): x [4, 2048, 1024] f32, qkv_w [1024, 3072], proj_w
[1024, 1024], proj_b [1024], 16 heads of dim 64, causal softmax.

Sharding: core c handles batch b = c // 2 and head-half c % 2 (8 of the 16
heads). Each core computes the QKV projection for its 8 heads, causal
attention, and the partial output projection (its 512 rows of proj_w). The
host sums the two partials per batch and adds the bias.

All matmul operands are bf16 (accumulation in f32 PSUM): halves DMA bytes
and enables FWL so LDWEIGHTS hides under streaming. The host pre-permutes
each weight so it lands in SBUF with a single large contiguous DMA.

On-core dataflow (head-dim on partitions everywhere):
  qT/kT = W.T @ x.T  (bf16 matmuls, stored bf16)            [128, N] per pair
  v     = x @ Wv     (bf16, stored bf16, k-rows on parts)   [N, 512+ones]
  scoresT[k,q] per head = kT-slice.T @ qT   (row-tiled e0/e1 concurrent)
  expT  = exp(0.125 * scoresT) on ACT, tri-mask on the diagonal 128-block
  outT_unnorm[dh,q] += v-slice.T @ expT     (accumulated over k chunks)
  sums ride the v ones-column -> sel-matmul broadcast -> fast reciprocal
  outT  = po * recip fused on DVE (divide folded into the PSUM->SBUF copy)
  partial = outT.T @ proj_w (bf16)

Scheduling: the attention inner loop is paced by the ACT engine (exp), so
scores are emitted one k-block ahead of the AV matmuls (ACT never starves
behind the in-order PE queue), and the PE bubbles are filled with the next
pair's QKV-projection matmuls / the output projection, emitted as small
4-matmul closures one per k-block iteration.
"""

import numpy as np

P = 128
N = 2048
D = 1024
DH = 512          # head dims per core (8 heads x 64)
HD = 64
HP = HD + 1       # head dims + ones column (softmax denominator row)
DHP = 8 * HP      # per-row-chunk v columns incl. ones (520)
NPAIR = 4
DC = D // P       # 8 contraction chunks
NRC = N // P      # 16 row chunks
NQC = N // 512    # 4 query 512-chunks

_CACHE = {}


def _build_nc(reps=1):
    from collections import deque
    from contextlib import ExitStack

    import concourse.bacc as bacc
    import concourse.tile as tile
    from concourse import mybir

    f32 = mybir.dt.float32
    bf16 = mybir.dt.bfloat16
    AF = mybir.ActivationFunctionType

    nc = bacc.Bacc("TRN2", target_bir_lowering=False, debug=False,
                   enable_asserts=False, num_devices=8)

    # host-side layouts (see make_in_maps):
    #   xt  [128, 16 * 1024]: block (qw, dc) of 512 cols = x.T[dc-rows, qw-cols]
    #   wv/wq/wk [128, 8 * 512]: block dc = W[dc-rows, :]
    #   pw  [128, 4 * 1024]: block pp = proj_w[pp-rows, :]
    xt = nc.dram_tensor("xt", [P, 4 * 4096], bf16, kind="ExternalInput").ap()
    wq = nc.dram_tensor("wq", [P, DC * DH], bf16, kind="ExternalInput").ap()
    wk = nc.dram_tensor("wk", [P, DC * DH], bf16, kind="ExternalInput").ap()
    wv = nc.dram_tensor("wv", [P, DC * DH], bf16, kind="ExternalInput").ap()
    pw = nc.dram_tensor("pw", [P, NPAIR * D], bf16, kind="ExternalInput").ap()
    tri = nc.dram_tensor("tri", [P, P], bf16, kind="ExternalInput").ap()
    sel = nc.dram_tensor("sel", [P, P], bf16, kind="ExternalInput").ap()
    out = nc.dram_tensor("out", [N, D], f32, kind="ExternalOutput").ap()

    def emit_rep(tc, const_tiles):
        tri_b, sel_b = const_tiles
        with ExitStack() as rep:
            big_ps = rep.enter_context(
                tc.tile_pool(name="big_ps", bufs=2, space="PSUM"))
            small_ps = rep.enter_context(
                tc.tile_pool(name="small_ps", bufs=2, space="PSUM"))
            po_ps = rep.enter_context(
                tc.tile_pool(name="po_ps", bufs=1, space="PSUM"))
            outT_pool = rep.enter_context(tc.tile_pool(name="outT", bufs=4))
            outT = [outT_pool.tile([P, N], bf16, name=f"outT{p}", tag="outT")
                    for p in range(NPAIR)]
            pw_pool = rep.enter_context(tc.tile_pool(name="pw", bufs=1))
            osb = rep.enter_context(tc.tile_pool(name="osb", bufs=4))
            pw_sb = pw_pool.tile([P, NPAIR * D], bf16)

            with ExitStack() as mid:
                xt_pool = mid.enter_context(tc.tile_pool(name="xt", bufs=1))
                xt_sb = xt_pool.tile([P, 4 * 4096], bf16)

                def xts(dc, q0, w):
                    # columns q0:q0+w of x.T row-block dc; must not cross a
                    # 512-col boundary
                    qw, r = q0 // 512, q0 % 512
                    base = qw * 4096 + dc * 512 + r
                    return xt_sb[:, base:base + w]

                vt_pool = mid.enter_context(tc.tile_pool(name="vt", bufs=1))
                # per head: 64 v-columns + a ones column, so the AV matmul's
                # 65th output partition accumulates the softmax denominator
                v_sb = vt_pool.tile([P, NRC * DHP], bf16)
                w_pool = mid.enter_context(tc.tile_pool(name="wqkv", bufs=3))
                wv_sb = w_pool.tile([P, DC * DH], bf16, tag="wv")
                wq_sb = w_pool.tile([P, DC * DH], bf16, tag="wq")
                wk_sb = w_pool.tile([P, DC * DH], bf16, tag="wk")

                # weights on the sync queue, xt on the Act queue, pw/consts
                # on the gpsimd queue. The first pieces are fine-grained so
                # phase B2's first matmuls wait on ~128KB, not ~1MB.
                # inputs split across the SP and Act DMA queues (~160GB/s
                # each), ordered by when phase B2 / the qkT chunks need them
                nc.sync.dma_start(wv_sb[:, 0:DH], wv[:, 0:DH])
                nc.sync.dma_start(wv_sb[:, DH:DC * DH], wv[:, DH:DC * DH])
                for dc in range(DC):
                    nc.scalar.dma_start(
                        xt_sb[:, dc * 512:(dc + 1) * 512],
                        xt[:, dc * 512:(dc + 1) * 512])
                nc.sync.dma_start(xt_sb[:, 4096:8192], xt[:, 4096:8192])
                nc.scalar.dma_start(xt_sb[:, 8192:12288], xt[:, 8192:12288])
                nc.sync.dma_start(wq_sb[:], wq)
                nc.scalar.dma_start(xt_sb[:, 12288:16384],
                                    xt[:, 12288:16384])
                nc.sync.dma_start(wk_sb[:], wk)
                nc.gpsimd.dma_start(pw_sb[:], pw)

                # ---- Phase B2: v = x @ Wv (k-rows on partitions), bf16 ----
                for rc in range(NRC):
                    pv = small_ps.tile([P, DH], f32, tag="sp")
                    for dc in range(DC):
                        nc.tensor.matmul(
                            pv[:],
                            xts(dc, rc * P, P),
                            wv_sb[:, dc * DH:(dc + 1) * DH],
                            start=(dc == 0), stop=(dc == DC - 1))
                    dst = v_sb[:, rc * DHP:(rc + 1) * DHP].rearrange(
                        "p (h c) -> p h c", h=8)
                    nc.vector.tensor_copy(
                        dst[:, :, 0:HD],
                        pv[:].rearrange("p (h c) -> p h c", h=8))
                    nc.vector.memset(dst[:, :, HD:HP], 1.0)

                qkT = mid.enter_context(tc.tile_pool(name="qkT", bufs=2))
                expp = mid.enter_context(tc.tile_pool(name="expp", bufs=5))
                ssbp = mid.enter_context(tc.tile_pool(name="ssb", bufs=2))
                drc_pool = mid.enter_context(tc.tile_pool(name="drc", bufs=1))
                # two persistent divisor-staging tiles; rows 0/32 are
                # rewritten with the raw softmax sums each round, other rows
                # only need to be non-NaN for the sel matmul
                ssb_t = []
                for i in range(2):
                    t2 = ssbp.tile([P, 512], bf16, name=f"ssbt{i}", tag="ssb")
                    nc.vector.memset(t2[0:HD, :], 1.0)
                    ssb_t.append(t2)

                def qkT_chunk_closures(p, qc, wt, dstT):
                    """One 512-col chunk of the q or k projection for pair p,
                    split into two 4-matmul filler closures (the second also
                    casts PSUM -> SBUF)."""
                    state = {}

                    def first():
                        state["ps"] = small_ps.tile([P, 512], f32,
                                                    name="fqk", tag="sp")
                        for dc in range(4):
                            nc.tensor.matmul(
                                state["ps"][:],
                                wt[:, dc * DH + p * P: dc * DH + (p + 1) * P],
                                xts(dc, qc * 512, 512),
                                start=(dc == 0), stop=False,
                                skip_group_check=True)

                    def second():
                        for dc in range(4, DC):
                            nc.tensor.matmul(
                                state["ps"][:],
                                wt[:, dc * DH + p * P: dc * DH + (p + 1) * P],
                                xts(dc, qc * 512, 512),
                                start=False, stop=(dc == DC - 1),
                                skip_group_check=True)
                        nc.vector.tensor_copy(
                            dstT[:, qc * 512:(qc + 1) * 512], state["ps"][:])

                    return [first, second]

                def proj_closure(rc, cc):
                    """Output projection for one [128, 512] tile: 4
                    accumulation matmuls + PSUM cast + store."""
                    def go():
                        pr = small_ps.tile([P, 512], f32, name="pr", tag="sp")
                        for pp in range(NPAIR):
                            nc.tensor.matmul(
                                pr[:],
                                outT[pp][:, rc * P:(rc + 1) * P],
                                pw_sb[:, pp * D + cc * 512:
                                      pp * D + (cc + 1) * 512],
                                start=(pp == 0), stop=(pp == NPAIR - 1))
                        ot = osb.tile([P, 512], f32, name="ot", tag="osb")
                        nc.vector.tensor_copy(ot[:], pr[:])
                        nc.gpsimd.dma_start(
                            out[rc * P:(rc + 1) * P,
                                cc * 512:(cc + 1) * 512], ot[:])
                    return go

                # ---- Attention: one flat global (pair, q-window, k-block)
                # stream. Scores run one step ahead and AV two steps behind
                # the exp stream, so a blocked AV (waiting for the po bank
                # to drain at a window tail) never starves the ACT engine.
                # Pair 0's qkT chunks are window-gated fillers: window w's
                # scores only need chunks qc <= w, so its attention starts
                # during its own projection.
                qkt = {0: (qkT.tile([P, N], bf16, name="qT0", tag="qT"),
                           qkT.tile([P, N], bf16, name="kT0", tag="kT"))}
                fillq0 = deque()
                for qc in range(NQC):
                    fillq0.extend(qkT_chunk_closures(0, qc, wq_sb,
                                                     qkt[0][0]))
                    fillq0.extend(qkT_chunk_closures(0, qc, wk_sb,
                                                     qkt[0][1]))
                def ensure_p0(w):
                    # consume pair-0 closures until chunks qc <= w are out
                    while len(fillq0) > 16 - 4 * (w + 1):
                        fillq0.popleft()()
                gseq = [(p, qc4, kc) for p in range(NPAIR)
                        for qc4 in range(NQC) for kc in range(4 * qc4 + 4)]
                fillq = deque()
                po = [None, None]
                av_tog = [0]

                def emit_scores(p, qc4, kc):
                    qT, kT = qkt[p]
                    qoff = max(0, kc * P - qc4 * 512)
                    q0 = qc4 * 512 + qoff
                    q1 = (qc4 + 1) * 512
                    ps_s = big_ps.tile([P, 1024], f32, name="ps_s", tag="bp")
                    for e in range(2):
                        nc.tensor.matmul(
                            ps_s[:, e * 512 + qoff: e * 512 + 512],
                            kT[e * HD:(e + 1) * HD, kc * P:(kc + 1) * P],
                            qT[e * HD:(e + 1) * HD, q0:q1],
                            start=True, stop=True)
                    return ps_s, qoff

                def do_av(et, qoff, p, qc4, kc):
                    nkc = 4 * qc4 + 4
                    if kc == 0:
                        po[0] = po_ps.tile([HP, 512], f32, name="po0",
                                           tag="po0")
                        po[1] = po_ps.tile([HP, 512], f32, name="po1",
                                           tag="po1")
                    for e in range(2):
                        h = 2 * p + e
                        nc.tensor.matmul(
                            po[e][0:HP, qoff:512],
                            v_sb[:, kc * DHP + h * HP:
                                 kc * DHP + (h + 1) * HP],
                            et[:, e * 512 + qoff: e * 512 + 512],
                            start=(kc == 0), stop=(kc == nkc - 1),
                            skip_group_check=True)
                    # fillers every other k-block (the PE has ~40% headroom
                    # under the exp stream), but greedily in the last pair
                    # (PE-bound there: attention + output projection)
                    av_tog[0] ^= 1
                    if p == NPAIR - 1 or av_tog[0]:
                        q = fillq0 if fillq0 else fillq
                        if q:
                            q.popleft()()
                    if kc == nkc - 1:
                        # q-window tail: drain po quickly (sums + raw
                        # copies), then the divisor chain and one in-place
                        # normalize of the outT slice
                        qs = slice(qc4 * 512, (qc4 + 1) * 512)
                        ssb = ssb_t[(p * NQC + qc4) % 2]
                        nc.vector.tensor_copy(ssb[0:1, :], po[0][HD:HP, :])
                        nc.vector.tensor_copy(ssb[32:33, :], po[1][HD:HP, :])
                        nc.vector.tensor_copy(outT[p][0:HD, qs],
                                              po[0][0:HD, :])
                        nc.vector.tensor_copy(outT[p][HD:P, qs],
                                              po[1][0:HD, :])
                        dps = small_ps.tile([P, 512], f32, name="dps",
                                            tag="sp")
                        nc.tensor.matmul(dps[:], sel_b[0:33, :],
                                         ssb[0:33, :], start=True, stop=True)
                        drc = drc_pool.tile([P, 512], f32, tag="drc")
                        nc.vector.reciprocal_approx_fast(drc[:], dps[:])
                        nc.vector.tensor_mul(outT[p][:, qs],
                                             outT[p][:, qs], drc[:])
                        if p == NPAIR - 1:
                            for rc in range(4 * qc4, 4 * qc4 + 4):
                                for cc in range(2):
                                    fillq.append(proj_closure(rc, cc))
                        q = fillq0 if fillq0 else fillq
                        if q:
                            q.popleft()()

                ensure_p0(0)
                sc_fifo = deque([emit_scores(*gseq[0])])
                pend = deque()
                cur_pair = -1
                for gi, it in enumerate(gseq):
                    p, qc4, kc = it
                    if p != cur_pair:
                        cur_pair = p
                        if p + 1 < NPAIR:
                            qkt[p + 1] = (
                                qkT.tile([P, N], bf16, name="qTn", tag="qT"),
                                qkT.tile([P, N], bf16, name="kTn", tag="kT"))
                            for qc in range(NQC):
                                fillq.extend(qkT_chunk_closures(
                                    p + 1, qc, wq_sb, qkt[p + 1][0]))
                                fillq.extend(qkT_chunk_closures(
                                    p + 1, qc, wk_sb, qkt[p + 1][1]))
                    if gi + 1 < len(gseq):
                        np_, nqc4, nkc_ = gseq[gi + 1]
                        if np_ != p:
                            # next pair's scores need its qkT casts emitted
                            while fillq0:
                                fillq0.popleft()()
                            while fillq:
                                fillq.popleft()()
                        elif np_ == 0 and nkc_ == 0:
                            ensure_p0(nqc4)
                        sc_fifo.append(emit_scores(*gseq[gi + 1]))
                    ps_s, qoff = sc_fifo.popleft()
                    et = expp.tile([P, 1024], bf16, name="et", tag="et")
                    ev = et[:].rearrange(
                        "p (h q) -> p h q", h=2)[:, :, qoff:512]
                    pv_ = ps_s[:].rearrange(
                        "p (h q) -> p h q", h=2)[:, :, qoff:512]
                    nc.scalar.activation(ev, pv_, AF.Exp, scale=0.125)
                    if kc >= 4 * qc4:  # diagonal block: causal mask
                        em = et[:].rearrange("p (a q) -> p a q", a=2)[
                            :, :, qoff:qoff + P]
                        trib = tri_b[:].rearrange(
                            "p (a q) -> p a q", a=1).broadcast_to([P, 2, P])
                        nc.vector.tensor_mul(em, em, trib)
                    pend.append((et, qoff) + it)
                    if len(pend) > 2:
                        do_av(*pend.popleft())
                while pend:
                    do_av(*pend.popleft())
                while fillq0:
                    fillq0.popleft()()
                while fillq:
                    fillq.popleft()()

    with tile.TileContext(nc) as tc, ExitStack() as ctx:
        const = ctx.enter_context(tc.tile_pool(name="const", bufs=1))
        tri_b = const.tile([P, P], bf16)
        nc.gpsimd.dma_start(tri_b[:], tri)
        sel_b = const.tile([P, P], bf16)
        nc.gpsimd.dma_start(sel_b[:], sel)
        const_tiles = (tri_b, sel_b)
        for _rep in range(reps):
            emit_rep(tc, const_tiles)

    nc.compile()
    return nc


def get_nc(reps=1):
    key = f"nc{reps}"
    if key not in _CACHE:
        _CACHE[key] = _build_nc(reps=reps)
    return _CACHE[key]


def _make_runner(nc, n_cores=8):
    """Cached jit over the bass_exec primitive (mirrors
    bass2jax.run_bass_via_pjrt's multi-core path, but reusable across calls
    so jax does not re-trace per invocation)."""
    import jax
    from jax.sharding import Mesh, PartitionSpec
    from jax.experimental.shard_map import shard_map
    from concourse import bass2jax, mybir

    bass2jax.install_neuronx_cc_hook()
    part_name = nc.partition_id_tensor.name if nc.partition_id_tensor else None
    in_names, out_names, out_avals, zero_templates = [], [], [], []
    for alloc in nc.m.functions[0].allocations:
        if not isinstance(alloc, mybir.MemoryLocationSet):
            continue
        name = alloc.memorylocations[0].name
        if alloc.kind == "ExternalInput":
            if name != part_name:
                in_names.append(name)
        elif alloc.kind == "ExternalOutput":
            out_names.append(name)
            shape = tuple(alloc.tensor_shape)
            dtype = mybir.dt.np(alloc.dtype)
            out_avals.append(jax.core.ShapedArray(shape, dtype))
            zero_templates.append((shape, dtype))
    n_params = len(in_names)
    n_outs = len(out_avals)
    all_names = in_names + out_names + ([part_name] if part_name else [])

    def _body(*args):
        operands = list(args)
        if part_name:
            operands.append(bass2jax.partition_id_tensor())
        outs = bass2jax._bass_exec_p.bind(
            *operands,
            out_avals=tuple(out_avals),
            in_names=tuple(all_names),
            out_names=tuple(out_names),
            lowering_input_output_aliases=(),
            sim_require_finite=True,
            sim_require_nnan=True,
            nc=nc,
        )
        return tuple(outs)

    devices = jax.devices()[:n_cores]
    mesh = Mesh(np.asarray(devices), ("core",))
    in_specs = (PartitionSpec("core"),) * (n_params + n_outs)
    out_specs = (PartitionSpec("core"),) * n_outs
    donate = tuple(range(n_params, n_params + n_outs))
    sharded = jax.jit(
        shard_map(_body, mesh=mesh, in_specs=in_specs, out_specs=out_specs,
                  check_rep=False),
        donate_argnums=donate, keep_unused=True)

    def run(in_maps):
        concat_in = [
            np.concatenate([np.asarray(m[name]) for m in in_maps], axis=0)
            for name in in_names
        ]
        concat_zeros = [
            np.zeros((n_cores * s[0], *s[1:]), d) for s, d in zero_templates
        ]
        out_arrs = sharded(*concat_in, *concat_zeros)
        return {
            name: np.asarray(out_arrs[i]).reshape(n_cores, *zero_templates[i][0])
            for i, name in enumerate(out_names)
        }

    run.sharded = sharded
    run.mesh = mesh
    run.in_names = in_names
    run.out_names = out_names
    run.zero_templates = zero_templates
    run.n_cores = n_cores
    return run


def get_runner(reps=1):
    key = f"runner{reps}"
    if key not in _CACHE:
        _CACHE[key] = _make_runner(get_nc(reps=reps))
    return _CACHE[key]


def _fold_rows(a):
    """[8*128, C] -> [128, 8*C]: row-block dc becomes column-block dc."""
    dcn, c = a.shape[0] // P, a.shape[1]
    return np.ascontiguousarray(
        a.reshape(dcn, P, c).transpose(1, 0, 2).reshape(P, dcn * c))


def make_in_maps(x, qkv_w, proj_w):
    import ml_dtypes
    bf = ml_dtypes.bfloat16
    x = np.asarray(x, dtype=np.float32)
    qkv_w = np.asarray(qkv_w, dtype=bf)
    proj_w = np.asarray(proj_w, dtype=bf)
    tri = np.triu(np.ones((P, P), dtype=bf))
    sel = np.zeros((P, P), dtype=bf)
    sel[0, 0:64] = 1.0
    sel[32, 64:128] = 1.0
    in_maps = []
    for c in range(8):
        b, half = c // 2, c % 2
        hs = half * DH
        xtb = x[b].T.astype(bf)  # [1024, 2048]
        # [p, (qw dc c)]: 512-col chunk of x.T row-block dc, query window qw
        xtr = np.ascontiguousarray(
            xtb.reshape(DC, P, 4, 512).transpose(1, 2, 0, 3).reshape(
                P, 4 * 4096))
        in_maps.append({
            "xt": xtr,
            "wq": _fold_rows(qkv_w[:, hs:hs + DH]),
            "wk": _fold_rows(qkv_w[:, D + hs:D + hs + DH]),
            "wv": _fold_rows(qkv_w[:, 2 * D + hs:2 * D + hs + DH]),
            "pw": _fold_rows(proj_w[hs:hs + DH, :]),
            "tri": tri,
            "sel": sel,
        })
    return in_maps


def kernel(x, qkv_w, proj_w, proj_b, **_):
    proj_b = np.asarray(proj_b, dtype=np.float32)
    run = get_runner()
    in_maps = make_in_maps(x, qkv_w, proj_w)
    parts = run(in_maps)["out"]
    outp = np.empty((4, N, D), dtype=np.float32)
    for b in range(4):
        outp[b] = parts[2 * b] + parts[2 * b + 1] + proj_b[None, :]
    return outp
